# revision 1
# baseline (speedup 1.0000x reference)
"""AllAtomFAPE loss on 8 TRN2 NeuronCores.

Strategy: dist^2[f,a] (+eps, masked) is a bilinear form over per-frame and
per-atom features:

    m_a^2 * (dist^2[f,a] + EPS) = sum_k A[a,k] * B[f,k],  K = 34

with A built from atom positions (pp/qq/pq outer products, p, q, 1) and B
from frame Gram matrices (Gp=RpRp^T, Gt=RtRt^T, M=RpRt^T, vectors, const).
The atom mask m^2 is folded into A (sqrt(m^2 x) = m sqrt(x)), EPS into B's
const row. Clamp folds through the sqrt: min(sqrt(y),10) = sqrt(min(y,100+EPS))
and max(y,0) guards rounding-induced negatives.

Frames (3072) are sharded across 8 cores (384 each); atoms replicated.
Per core: matmul (PE, bf16) -> clamp (DVE) -> sqrt + free-axis accumulate
(ACT accum_out, frame mask folded into the per-partition sqrt scale) ->
per-partition partial sums DMA'd out. Host sums the partials and applies
1/(atom_count*frame_count*Z). Masks are assumed 0/1 (AlphaFold semantics).
"""
import numpy as np
import ml_dtypes

import concourse.bass as bass
from concourse import bacc, tile, mybir
from concourse.bass_utils import run_bass_kernel_spmd

D_CLAMP = 10.0
EPS = 1e-4
Z = 10.0

B_, N_, F_, A_ = 1, 384, 8, 14
NF = N_ * F_            # 3072 frames total
NA = N_ * A_            # 5376 atoms
NCORES = 8
NF_LOC = NF // NCORES   # 384 frames per core
K = 34                  # bilinear contraction dim
FT = NF_LOC // 128      # 3 frame tiles per core
CHUNK = 512             # matmul free-dim cap

_cache = {}


# PSUM tiles: up to 2048 f32 = 4 banks (x2 bufs = all 8); matmul free-dim
# cap is 512. DVE clamps each PSUM tile into a big SBUF strip; ACT sqrt runs
# over big SBUF spans to amortize its ~352-cycle pipe + 187ns
# accumulator-read per-instruction overheads.
#  - ft0 leads with a small chunk so the DVE pipeline starts early
#  - the last ft tapers (2048/1792/1536) so the final sqrt tail is short
DEFAULT_CFG = dict(   # TimelineSim-tuned chunk schedule
    ptiles_ft=[
        [(0, 640), (640, 1408), (2048, 2048), (4096, 1280)],
        [(0, 2048), (2048, 2048), (4096, 1280)],
        [(0, 2048), (2048, 1792), (3840, 1536)],
    ],
    act_ft=[
        [(0, 2048), (2048, 2048), (4096, 1280)],
        [(0, 2048), (2048, 2048), (4096, 1280)],
        [(0, 2048), (2048, 1792), (3840, 1536)],
    ],
    dma_stages=[(0, 640), (640, 1408), (2048, 2048), (4096, 1280)],
)


def _build_graph(cfg=None):
    cfg = cfg or DEFAULT_CFG
    nc = bacc.Bacc("TRN2", target_bir_lowering=False, debug=False)

    bf16 = mybir.dt.bfloat16
    f32 = mybir.dt.float32

    ptiles_ft = cfg["ptiles_ft"]
    act_ft = cfg["act_ft"]
    PW = max(cw for ft in ptiles_ft for (_, cw) in ft)
    assert PW <= 2048
    n_cols = sum(len(a) for a in act_ft)
    pcol = 0

    # pk packs bT (cols 0:NF_LOC) and A^T (cols NF_LOC:NF_LOC+NA) so the
    # first DMA delivers the stationary weights together with the first atom
    # chunk (one HWDGE transaction instead of two serialized ones).
    pk_d = nc.dram_tensor("pk", (K, NF_LOC + NA), bf16, kind="ExternalInput")
    fm_d = nc.dram_tensor("fm", (128, FT), f32, kind="ExternalInput")
    out_d = nc.dram_tensor("out", (128, n_cols), f32, kind="ExternalOutput")

    with tile.TileContext(nc) as tc:
        with (
            tc.tile_pool(name="const", bufs=1) as const,
            tc.tile_pool(name="big", bufs=2) as big,
            tc.tile_pool(name="psum", bufs=2, space="PSUM") as psum,
        ):
            pk = const.tile([K, NF_LOC + NA], bf16)
            fm = const.tile([128, FT], f32)
            partials = const.tile([128, n_cols], f32)

            # preload the ACT sqrt table during the input-DMA window so its
            # ~1.3us load doesn't count against ACT's work window
            warm = const.tile([128, 1], f32)
            nc.vector.memset(warm[:], 1.0)
            nc.scalar.sqrt(warm[:], warm[:])

            # fm goes via the Pool engine's SWDGE path so it doesn't
            # serialize with pk on the single HWDGE queue
            nc.gpsimd.dma_start(out=fm[:], in_=fm_d[:])
            # stage the big pk load so the first matmuls start early; the
            # first slice carries bT plus the first atom columns
            for i, (a0, aw) in enumerate(cfg["dma_stages"]):
                c0 = 0 if i == 0 else NF_LOC + a0
                cw = NF_LOC + aw if i == 0 else aw
                nc.sync.dma_start(out=pk[:, c0:c0 + cw], in_=pk_d[:, c0:c0 + cw])

            for ft in range(FT):
                cl_big = big.tile([128, NA], f32, tag="cl")
                dummy = big.tile([128, 4096], bf16, tag="dummy")
                for (c0, cw) in ptiles_ft[ft]:
                    d2 = psum.tile([128, PW], f32)
                    for s0 in range(0, cw, CHUNK):
                        sw = min(CHUNK, cw - s0)
                        nc.tensor.matmul(
                            d2[:, s0:s0 + sw],
                            pk[:, ft * 128:(ft + 1) * 128],
                            pk[:, NF_LOC + c0 + s0:NF_LOC + c0 + s0 + sw],
                            start=True, stop=True,
                        )
                    nc.vector.tensor_scalar(
                        cl_big[:, c0:c0 + cw], d2[:, :cw],
                        0.0, 100.0 + EPS,
                        op0=mybir.AluOpType.max, op1=mybir.AluOpType.min,
                    )
                # sqrt(fm * y) = fm * sqrt(y) for 0/1 frame masks: the
                # per-partition scale folds the frame weighting into the
                # accumulation for free.
                for (a0, aw) in act_ft[ft]:
                    nc.scalar.activation(
                        dummy[:, :aw], cl_big[:, a0:a0 + aw],
                        mybir.ActivationFunctionType.Sqrt,
                        scale=fm[:, ft:ft + 1],
                        accum_out=partials[:, pcol:pcol + 1],
                    )
                    pcol += 1

            # final cross-partition/core reduction happens on the host:
            # just ship the (128, n) accumulator columns back
            nc.sync.dma_start(out=out_d[:, :pcol], in_=partials[:, :pcol])

    nc.compile()
    nc.finalize()
    return nc


def _features(predicted_frames_R, predicted_frames_t, predicted_atom_positions,
              atom_mask, true_frames_R, true_frames_t, true_atom_positions,
              seq_mask):
    """Host-side O(N+F) feature build. Returns A (NA,K), B (NF,K), fm, counts."""
    f32 = np.float32
    Rp = np.asarray(predicted_frames_R, f32).reshape(NF, 3, 3)
    tp = np.asarray(predicted_frames_t, f32).reshape(NF, 3)
    Rt = np.asarray(true_frames_R, f32).reshape(NF, 3, 3)
    tt = np.asarray(true_frames_t, f32).reshape(NF, 3)
    p = np.asarray(predicted_atom_positions, f32).reshape(NA, 3)
    q = np.asarray(true_atom_positions, f32).reshape(NA, 3)
    m = (np.asarray(atom_mask, f32) * np.asarray(seq_mask, f32)[:, :, None]).reshape(NA)
    fm = np.broadcast_to(
        np.asarray(seq_mask, f32)[:, :, None], (B_, N_, F_)).reshape(NF).copy()

    pp = np.einsum('aj,ak->ajk', p, p).reshape(NA, 9)
    qq = np.einsum('aj,ak->ajk', q, q).reshape(NA, 9)
    pq = np.einsum('aj,ak->ajk', p, q).reshape(NA, 9)
    Afeat = np.concatenate(
        [pp, qq, pq, p, q, np.ones((NA, 1), f32)], axis=1) * (m ** 2)[:, None]

    Gp = np.einsum('fij,fkj->fik', Rp, Rp)
    Gt = np.einsum('fij,fkj->fik', Rt, Rt)
    M = np.einsum('fij,fkj->fik', Rp, Rt)
    vec_p = -2 * np.einsum('fjk,fk->fj', Gp, tp) + 2 * np.einsum('fjk,fk->fj', M, tt)
    vec_q = -2 * np.einsum('fjk,fk->fj', Gt, tt) + 2 * np.einsum('fkj,fk->fj', M, tp)
    const = (np.einsum('fj,fjk,fk->f', tp, Gp, tp)
             + np.einsum('fj,fjk,fk->f', tt, Gt, tt)
             - 2 * np.einsum('fj,fjk,fk->f', tp, M, tt) + EPS)
    Bfeat = np.concatenate(
        [Gp.reshape(NF, 9), Gt.reshape(NF, 9), -2 * M.reshape(NF, 9),
         vec_p, vec_q, const[:, None]], axis=1)

    ac = max(float(m.sum()), 1.0)
    fc = max(float(fm.sum()), 1.0)
    return Afeat, Bfeat, fm, ac, fc


def make_in_maps(inputs):
    Afeat, Bfeat, fm, ac, fc = _features(**inputs)
    bf16 = ml_dtypes.bfloat16
    aT = Afeat.T.astype(bf16)                                  # (K, NA)
    in_maps = []
    for c in range(NCORES):
        Bc = Bfeat[c * NF_LOC:(c + 1) * NF_LOC]                # (NF_LOC, K)
        pk = np.concatenate([Bc.T.astype(bf16), aT], axis=1)   # (K, NF_LOC+NA)
        fmc = np.ascontiguousarray(
            fm[c * NF_LOC:(c + 1) * NF_LOC].reshape(FT, 128).T)  # (128, FT)
        in_maps.append({"pk": np.ascontiguousarray(pk), "fm": fmc})
    return in_maps, ac, fc


def _build_fast_exec(nc):
    """Cache the jitted 8-core executable so repeat kernel() calls skip jax
    re-tracing. Mirrors bass2jax.run_bass_via_pjrt's multi-core path."""
    import jax
    from concourse import bass2jax
    from jax.experimental.shard_map import shard_map
    from jax.sharding import Mesh, PartitionSpec

    bass2jax.install_neuronx_cc_hook()
    partition_name = nc.partition_id_tensor.name if nc.partition_id_tensor else None

    in_names, out_names, out_avals, zero_shapes = [], [], [], []
    for alloc in nc.m.functions[0].allocations:
        if not isinstance(alloc, mybir.MemoryLocationSet):
            continue
        name = alloc.memorylocations[0].name
        if alloc.kind == "ExternalInput":
            if name != partition_name:
                in_names.append(name)
        elif alloc.kind == "ExternalOutput":
            shape = tuple(alloc.tensor_shape)
            dtype = mybir.dt.np(alloc.dtype)
            out_names.append(name)
            out_avals.append(jax.core.ShapedArray(shape, dtype))
            zero_shapes.append((shape, dtype))
    n_params = len(in_names)
    all_names = in_names + out_names + ([partition_name] if partition_name else [])
    donate = tuple(range(n_params, n_params + len(out_names)))

    def _body(*args):
        operands = list(args)
        if partition_name is not None:
            operands.append(bass2jax.partition_id_tensor())
        return tuple(bass2jax._bass_exec_p.bind(
            *operands,
            out_avals=tuple(out_avals),
            in_names=tuple(all_names),
            out_names=tuple(out_names),
            lowering_input_output_aliases=(),
            sim_require_finite=True,
            sim_require_nnan=True,
            nc=nc,
        ))

    devices = jax.devices()[:NCORES]
    mesh = Mesh(np.asarray(devices), ("core",))
    specs = (PartitionSpec("core"),) * (n_params + len(out_names))
    sharded = jax.jit(
        shard_map(_body, mesh=mesh, in_specs=specs,
                  out_specs=(PartitionSpec("core"),) * len(out_names),
                  check_rep=False),
        donate_argnums=donate, keep_unused=True,
    )

    def run(in_maps):
        concat_in = [
            np.concatenate([np.asarray(m[k]) for m in in_maps], axis=0)
            for k in in_names
        ]
        concat_zeros = [
            np.zeros((NCORES * s[0], *s[1:]), dt) for (s, dt) in zero_shapes
        ]
        outs = sharded(*concat_in, *concat_zeros)
        return [
            {name: np.asarray(outs[i]).reshape(NCORES, *zero_shapes[i][0])[c]
             for i, name in enumerate(out_names)}
            for c in range(NCORES)
        ]

    return run


def kernel(**inputs) -> np.ndarray:
    in_maps, ac, fc = make_in_maps(inputs)

    if "nc" not in _cache:
        _cache["nc"] = _build_graph()
    nc = _cache["nc"]

    results = None
    try:
        if "fast" not in _cache:
            _cache["fast"] = _build_fast_exec(nc)
        results = _cache["fast"](in_maps)
    except Exception:
        _cache.pop("fast", None)
        results = run_bass_kernel_spmd(
            nc, in_maps, core_ids=list(range(NCORES))).results

    total = sum(float(r["out"].sum(dtype=np.float64)) for r in results)
    loss = total / (ac * fc * Z)
    return np.array([loss], np.float32)



# revision 2
# speedup vs baseline: 1.0114x; 1.0114x over previous
"""AllAtomFAPE loss on 8 TRN2 NeuronCores — ACT-floor pipeline.

dist^2[f,a] (+eps, masked) is a bilinear form over per-frame and per-atom
features:  m_a^2 * (dist^2[f,a] + EPS) = sum_k A[a,k] * B[f,k],  K = 34.
The frame mask is folded into B (fm in {0,1}), the atom mask into A.

Per core (384 frames = 3 tiles x 128, all 5376 atoms => 16128 cols):
  PE   : f32r matmuls (1 cyc/col for moving>=256) -> f32 PSUM, exact-enough
         that d2+EPS stays positive (no pre-sqrt clamp needed).
  ACT  : sqrt PSUM -> SBUF bf16 (no accum_out; 0.833 ns/col is the floor).
  DVE  : clamp min(s,10) then max(.,0), bf16 SBUF 4x mode (0.26 ns/col).
  PE   : 126 ones-matmuls (out free=1 -> ~1 cycle each) column-sum the
         clamped values into one PSUM bank; DMA [128,126] out; host sums.

PSUM: d2a [128,2048] (4 banks) + d2b [128,1536] (3) + acc (1) = 8 banks.
"""
import numpy as np

import concourse.bass as bass
from concourse import bacc, tile, mybir
from concourse.bass_utils import run_bass_kernel_spmd

D_CLAMP = 10.0
EPS = 1e-4
Z = 10.0

B_, N_, F_, A_ = 1, 384, 8, 14
NF = N_ * F_            # 3072 frames total
NA = N_ * A_            # 5376 atoms
NCORES = 8
NF_LOC = NF // NCORES   # 384 frames per core
K = 34                  # bilinear contraction dim
FT = NF_LOC // 128      # 3 frame tiles per core
STREAM = FT * NA        # 16128 columns per core
TA, TB = 2048, 1536     # alternating PSUM tile sizes (4 + 3 banks)
SQRT_BIAS = 1e-3        # safety bias inside sqrt( y + b ): guards rounding

_cache = {}

# Tunables (TimelineSim-swept)
DEFAULT_CFG = dict(
    # used width of each psum round; tags alternate a/b; widths <= slot size.
    # Small leading rounds start ACT early (the round-completion semaphore is
    # tile-level); tiny last round keeps the out-DMA gate short.
    round_widths=[1024, 1536, 2048, 1536, 2048, 1536, 2048, 1536, 2048, 768],
    # DVE clamp chunk boundaries over the 16128-wide s strip (must be on
    # round-cumsum boundaries and multiples of 128)
    dve_bounds=[0, 2560, 6144, 9728, 13312, 15360, 16128],
    # input DMA stages in atom columns (first stage also carries the 384 B cols)
    dma_stages=[1024, 2048, 2048, 256],
    # PE p-state warm-up: (count, free width) dummy matmuls into the acc bank
    pe_warm=(24, 120),
    # stream column from which clamped values ship raw (host-side sum)
    ship_from=15360,
)


def _round_layout(cfg):
    """Alternating psum rounds (tag, stream_start, used_width)."""
    rounds = []
    pos = 0
    for i, w in enumerate(cfg["round_widths"]):
        tag = "a" if i % 2 == 0 else "b"
        assert w <= (TA if tag == "a" else TB)
        rounds.append((tag, pos, w))
        pos += w
    assert pos == STREAM, pos
    return rounds


def _chunks(start, width):
    """Matmul chunks: split at 512 grid (psum-bank friendly) and frame-tile
    boundaries (lhsT changes every NA stream cols). All pieces >= 256."""
    out = []
    pos = start
    end = start + width
    while pos < end:
        nxt = min(end, pos + 512 - (pos - start) % 512)
        ft_b = (pos // NA + 1) * NA
        nxt = min(nxt, ft_b)
        out.append((pos, nxt - pos))
        pos = nxt
    return out


def _build_graph(cfg=None):
    cfg = cfg or DEFAULT_CFG
    nc = bacc.Bacc("TRN2", target_bir_lowering=False, debug=False)

    bf16 = mybir.dt.bfloat16
    f32 = mybir.dt.float32
    f32r = mybir.dt.float32r

    # columns below SHIP_FROM are reduced on-device (ones-matmuls); the tail
    # [SHIP_FROM:STREAM] ships as raw clamped bf16 and is summed on the host,
    # which drops two engine hops from the output gate.
    ship_from = cfg["ship_from"]
    NJ = ship_from // 128       # ones-reduce matmul count
    NC_TAIL = STREAM - ship_from

    pk_d = nc.dram_tensor("pk", (K, NF_LOC + NA), f32r, kind="ExternalInput")
    out_d = nc.dram_tensor("out", (128, NJ), f32, kind="ExternalOutput")
    outc_d = nc.dram_tensor("outc", (128, NC_TAIL), bf16, kind="ExternalOutput")

    rounds = _round_layout(cfg)

    with tile.TileContext(nc) as tc:
        with (
            tc.tile_pool(name="const", bufs=1) as const,
            tc.tile_pool(name="psum", bufs=1, space="PSUM") as psum,
        ):
            pk = const.tile([K, NF_LOC + NA], f32r)
            ones = const.tile([128, 1], bf16)
            warm = const.tile([128, 1], f32)
            bvec = const.tile([128, 1], f32)
            zeros = const.tile([128, 128 + cfg["pe_warm"][1]], bf16)
            s = const.tile([128, STREAM], bf16)
            c = const.tile([128, STREAM], bf16)
            accs = const.tile([128, NJ], f32)
            acc = psum.tile([128, NJ], f32, tag="acc")

            # zeros first: the PE warm-up below is gated on it
            nc.vector.memset(zeros[:], 0.0)
            # sqrt-table preload during the input-DMA window
            nc.vector.memset(warm[:], 1.0)
            nc.scalar.sqrt(warm[:], warm[:])
            nc.vector.memset(ones[:], 1.0)
            nc.vector.memset(bvec[:], SQRT_BIAS)

            # staged input DMA; stage 0 carries the B cols + first atoms
            a0 = 0
            for i, aw in enumerate(cfg["dma_stages"]):
                c0 = 0 if i == 0 else NF_LOC + a0
                cw = NF_LOC + aw if i == 0 else aw
                nc.sync.dma_start(out=pk[:, c0:c0 + cw], in_=pk_d[:, c0:c0 + cw])
                a0 += aw
            assert a0 == NA

            # PE p-state warm-up during the DMA window: zero matmuls into the
            # acc bank (every column is overwritten by the ones-reduce later).
            # ~3us of continuous PE busy reaches the 2.4 GHz p-state before
            # the first real matmul issues.
            n_warm, w_warm = cfg["pe_warm"]
            for _ in range(n_warm):
                nc.tensor.matmul(
                    acc[:, 0:w_warm],
                    zeros[0:K, 0:128],
                    zeros[0:K, 128:128 + w_warm],
                    start=True, stop=True,
                )

            # grid: PE matmuls + ACT sqrt spans, round-robin over a/b tiles
            for ri, (tag, start, w) in enumerate(rounds):
                size = TA if tag == "a" else TB
                d2 = psum.tile([128, size], f32, tag=tag)
                for (pos, cw) in _chunks(start, w):
                    ft = pos // NA
                    ac_ = pos % NA
                    nc.tensor.matmul(
                        d2[:, pos - start:pos - start + cw],
                        pk[:, ft * 128:(ft + 1) * 128],
                        pk[:, NF_LOC + ac_:NF_LOC + ac_ + cw],
                        start=True, stop=True,
                    )
                nc.scalar.activation(
                    s[:, start:start + w], d2[:, 0:w],
                    mybir.ActivationFunctionType.Sqrt,
                    bias=bvec[:, 0:1],
                )

            # DVE clamp: min(s,10) then max(.,0) at 4x (bf16 SBUF->SBUF)
            bounds = cfg["dve_bounds"]
            for r0, r1 in zip(bounds[:-1], bounds[1:]):
                nc.vector.tensor_scalar(
                    c[:, r0:r1], s[:, r0:r1],
                    D_CLAMP, 0.0,
                    op0=mybir.AluOpType.min, op1=mybir.AluOpType.max,
                )

            # column sums via 1-cycle ones-matmuls. The scheduler fence keeps
            # them AFTER every grid matmul in the in-order PE queue —
            # otherwise Tile hoists them next to their DVE deps and the
            # blocked ones-matmuls stall the rest of the grid.
            tc.no_sync_barrier()
            for j in range(NJ):
                nc.tensor.matmul(
                    acc[:, j:j + 1],
                    c[:, j * 128:(j + 1) * 128],
                    ones[:, 0:1],
                    start=True, stop=True,
                )

            # Output path 1 (SP queue): the raw clamped tail slice, gated only
            # on the last DVE clamp chunk.
            nc.sync.dma_start(out=outc_d[:, :], in_=c[:, ship_from:STREAM])
            # Output path 2 (ACT queue, idle by now): PSUM has no DMA port, so
            # hop the reduced columns through SBUF, then DMA.
            nc.scalar.copy(accs[:, :], acc[:, :])
            nc.scalar.dma_start(out=out_d[:, :], in_=accs[:, :])

    nc.compile()
    nc.finalize()
    return nc


def _features(predicted_frames_R, predicted_frames_t, predicted_atom_positions,
              atom_mask, true_frames_R, true_frames_t, true_atom_positions,
              seq_mask):
    """Host-side O(N+F) feature build. Returns A (NA,K), B (NF,K), counts."""
    f32 = np.float32
    Rp = np.asarray(predicted_frames_R, f32).reshape(NF, 3, 3)
    tp = np.asarray(predicted_frames_t, f32).reshape(NF, 3)
    Rt = np.asarray(true_frames_R, f32).reshape(NF, 3, 3)
    tt = np.asarray(true_frames_t, f32).reshape(NF, 3)
    p = np.asarray(predicted_atom_positions, f32).reshape(NA, 3)
    q = np.asarray(true_atom_positions, f32).reshape(NA, 3)
    m = (np.asarray(atom_mask, f32) * np.asarray(seq_mask, f32)[:, :, None]).reshape(NA)
    fm = np.broadcast_to(
        np.asarray(seq_mask, f32)[:, :, None], (B_, N_, F_)).reshape(NF)

    pp = np.einsum('aj,ak->ajk', p, p).reshape(NA, 9)
    qq = np.einsum('aj,ak->ajk', q, q).reshape(NA, 9)
    pq = np.einsum('aj,ak->ajk', p, q).reshape(NA, 9)
    Afeat = np.concatenate(
        [pp, qq, pq, p, q, np.ones((NA, 1), f32)], axis=1) * (m ** 2)[:, None]

    Gp = np.einsum('fij,fkj->fik', Rp, Rp)
    Gt = np.einsum('fij,fkj->fik', Rt, Rt)
    M = np.einsum('fij,fkj->fik', Rp, Rt)
    vec_p = -2 * np.einsum('fjk,fk->fj', Gp, tp) + 2 * np.einsum('fjk,fk->fj', M, tt)
    vec_q = -2 * np.einsum('fjk,fk->fj', Gt, tt) + 2 * np.einsum('fkj,fk->fj', M, tp)
    const = (np.einsum('fj,fjk,fk->f', tp, Gp, tp)
             + np.einsum('fj,fjk,fk->f', tt, Gt, tt)
             - 2 * np.einsum('fj,fjk,fk->f', tp, M, tt) + EPS)
    Bfeat = np.concatenate(
        [Gp.reshape(NF, 9), Gt.reshape(NF, 9), -2 * M.reshape(NF, 9),
         vec_p, vec_q, const[:, None]], axis=1)
    # fold the 0/1 frame mask into B: masked frames -> d2 == 0 -> dist 0
    Bfeat = Bfeat * fm[:, None]

    ac = max(float(m.sum()), 1.0)
    fc = max(float(fm.sum()), 1.0)
    return Afeat, Bfeat, ac, fc


def make_in_maps(inputs):
    Afeat, Bfeat, ac, fc = _features(**inputs)
    f32 = np.float32
    aT = np.ascontiguousarray(Afeat.T.astype(f32))            # (K, NA)
    in_maps = []
    for cix in range(NCORES):
        Bc = Bfeat[cix * NF_LOC:(cix + 1) * NF_LOC]           # (NF_LOC, K)
        pk = np.concatenate([Bc.T.astype(f32), aT], axis=1)   # (K, NF_LOC+NA)
        in_maps.append({"pk": np.ascontiguousarray(pk)})
    return in_maps, ac, fc


def _build_fast_exec(nc):
    """Cache the jitted 8-core executable so repeat kernel() calls skip jax
    re-tracing. Mirrors bass2jax.run_bass_via_pjrt's multi-core path."""
    import jax
    from concourse import bass2jax
    from jax.experimental.shard_map import shard_map
    from jax.sharding import Mesh, PartitionSpec

    bass2jax.install_neuronx_cc_hook()
    partition_name = nc.partition_id_tensor.name if nc.partition_id_tensor else None

    in_names, out_names, out_avals, zero_shapes = [], [], [], []
    for alloc in nc.m.functions[0].allocations:
        if not isinstance(alloc, mybir.MemoryLocationSet):
            continue
        name = alloc.memorylocations[0].name
        if alloc.kind == "ExternalInput":
            if name != partition_name:
                in_names.append(name)
        elif alloc.kind == "ExternalOutput":
            shape = tuple(alloc.tensor_shape)
            dtype = mybir.dt.np(alloc.dtype)
            out_names.append(name)
            out_avals.append(jax.core.ShapedArray(shape, dtype))
            zero_shapes.append((shape, dtype))
    n_params = len(in_names)
    all_names = in_names + out_names + ([partition_name] if partition_name else [])
    donate = tuple(range(n_params, n_params + len(out_names)))

    def _body(*args):
        operands = list(args)
        if partition_name is not None:
            operands.append(bass2jax.partition_id_tensor())
        return tuple(bass2jax._bass_exec_p.bind(
            *operands,
            out_avals=tuple(out_avals),
            in_names=tuple(all_names),
            out_names=tuple(out_names),
            lowering_input_output_aliases=(),
            sim_require_finite=True,
            sim_require_nnan=True,
            nc=nc,
        ))

    devices = jax.devices()[:NCORES]
    mesh = Mesh(np.asarray(devices), ("core",))
    specs = (PartitionSpec("core"),) * (n_params + len(out_names))
    sharded = jax.jit(
        shard_map(_body, mesh=mesh, in_specs=specs,
                  out_specs=(PartitionSpec("core"),) * len(out_names),
                  check_rep=False),
        donate_argnums=donate, keep_unused=True,
    )

    def run(in_maps):
        concat_in = [
            np.concatenate([np.asarray(m[k]) for m in in_maps], axis=0)
            for k in in_names
        ]
        concat_zeros = [
            np.zeros((NCORES * s[0], *s[1:]), dt) for (s, dt) in zero_shapes
        ]
        outs = sharded(*concat_in, *concat_zeros)
        return [
            {name: np.asarray(outs[i]).reshape(NCORES, *zero_shapes[i][0])[c]
             for i, name in enumerate(out_names)}
            for c in range(NCORES)
        ]

    return run


def kernel(**inputs) -> np.ndarray:
    in_maps, ac, fc = make_in_maps(inputs)

    if "nc" not in _cache:
        _cache["nc"] = _build_graph()
    nc = _cache["nc"]

    results = None
    try:
        if "fast" not in _cache:
            _cache["fast"] = _build_fast_exec(nc)
        results = _cache["fast"](in_maps)
    except Exception:
        _cache.pop("fast", None)
        results = run_bass_kernel_spmd(
            nc, in_maps, core_ids=list(range(NCORES))).results

    total = reduce_outputs(results)
    loss = total / (ac * fc * Z)
    return np.array([loss], np.float32)


def reduce_outputs(results) -> float:
    """Sum the per-core outputs: reduced columns + raw clamped tail slice."""
    total = 0.0
    for r in results:
        total += float(r["out"].sum(dtype=np.float64))
        total += float(np.asarray(r["outc"], np.float64).sum())
    return total


# revision 3
# speedup vs baseline: 1.0146x; 1.0031x over previous
"""AllAtomFAPE loss on 8 TRN2 NeuronCores — ACT-floor pipeline.

dist^2[f,a] (+eps, masked) is a bilinear form over per-frame and per-atom
features:  m_a^2 * (dist^2[f,a] + EPS) = sum_k A[a,k] * B[f,k],  K = 34.
The frame mask is folded into B (fm in {0,1}), the atom mask into A.

Per core (384 frames = 3 tiles x 128, all 5376 atoms => 16128 cols):
  PE   : f32r matmuls (1 cyc/col for moving>=256) -> f32 PSUM, exact-enough
         that d2+EPS stays positive (no pre-sqrt clamp needed).
  ACT  : sqrt PSUM -> SBUF bf16 (no accum_out; 0.833 ns/col is the floor).
  DVE  : clamp min(s,10) then max(.,0), bf16 SBUF 4x mode (0.26 ns/col).
  PE   : 126 ones-matmuls (out free=1 -> ~1 cycle each) column-sum the
         clamped values into one PSUM bank; DMA [128,126] out; host sums.

PSUM: d2a [128,2048] (4 banks) + d2b [128,1536] (3) + acc (1) = 8 banks.
"""
import numpy as np

import concourse.bass as bass
from concourse import bacc, tile, mybir
from concourse.bass_utils import run_bass_kernel_spmd

D_CLAMP = 10.0
EPS = 1e-4
Z = 10.0

B_, N_, F_, A_ = 1, 384, 8, 14
NF = N_ * F_            # 3072 frames total
NA = N_ * A_            # 5376 atoms
NCORES = 8
NF_LOC = NF // NCORES   # 384 frames per core
K = 34                  # bilinear contraction dim
FT = NF_LOC // 128      # 3 frame tiles per core
STREAM = FT * NA        # 16128 columns per core
TA, TB = 2048, 1536     # alternating PSUM tile sizes (4 + 3 banks)
SQRT_BIAS = 1e-3        # safety bias inside sqrt( y + b ): guards rounding

_cache = {}

# Tunables (TimelineSim-swept)
DEFAULT_CFG = dict(
    # used width of each psum round; tags alternate a/b; widths <= slot size.
    # Small leading rounds start ACT early (the round-completion semaphore is
    # tile-level); tiny last round keeps the out-DMA gate short.
    round_widths=[1024, 1536, 2048, 1536, 2048, 1536, 2048, 1536, 2048, 768],
    # DVE clamp chunk boundaries over the 16128-wide s strip (must be on
    # round-cumsum boundaries and multiples of 128)
    dve_bounds=[0, 2560, 6144, 9728, 13312, 15360, 16128],
    # input DMA stages in atom columns (first stage also carries the 384 B cols)
    dma_stages=[1024, 1536, 1536, 1280],
    # PE p-state warm-up: (count, free width) dummy matmuls into the acc bank.
    # Swept 0..22 — identical timeline (the cold-p-state first matmuls are off
    # the critical path), so keep it off.
    pe_warm=(0, 120),
    # stream column from which clamped values ship raw (host-side sum)
    ship_from=15360,
)


def _round_layout(cfg):
    """Alternating psum rounds (tag, stream_start, used_width)."""
    rounds = []
    pos = 0
    for i, w in enumerate(cfg["round_widths"]):
        tag = "a" if i % 2 == 0 else "b"
        assert w <= (TA if tag == "a" else TB)
        rounds.append((tag, pos, w))
        pos += w
    assert pos == STREAM, pos
    return rounds


def _chunks(start, width):
    """Matmul chunks: split at 512 grid (psum-bank friendly) and frame-tile
    boundaries (lhsT changes every NA stream cols). All pieces >= 256."""
    out = []
    pos = start
    end = start + width
    while pos < end:
        nxt = min(end, pos + 512 - (pos - start) % 512)
        ft_b = (pos // NA + 1) * NA
        nxt = min(nxt, ft_b)
        out.append((pos, nxt - pos))
        pos = nxt
    return out


def _build_graph(cfg=None):
    cfg = cfg or DEFAULT_CFG
    nc = bacc.Bacc("TRN2", target_bir_lowering=False, debug=False)

    bf16 = mybir.dt.bfloat16
    f32 = mybir.dt.float32
    f32r = mybir.dt.float32r

    # columns below SHIP_FROM are reduced on-device (ones-matmuls); the tail
    # [SHIP_FROM:STREAM] ships as raw clamped bf16 and is summed on the host,
    # which drops two engine hops from the output gate.
    ship_from = cfg["ship_from"]
    NJ = ship_from // 128       # ones-reduce matmul count
    NC_TAIL = STREAM - ship_from

    pk_d = nc.dram_tensor("pk", (K, NF_LOC + NA), f32r, kind="ExternalInput")
    # 128 cols (512B rows): cols NJ..127 are zero-padding, summed harmlessly
    out_d = nc.dram_tensor("out", (128, 128), f32, kind="ExternalOutput")
    outc_d = nc.dram_tensor("outc", (128, NC_TAIL), bf16, kind="ExternalOutput")

    rounds = _round_layout(cfg)

    with tile.TileContext(nc) as tc:
        with (
            tc.tile_pool(name="const", bufs=1) as const,
            tc.tile_pool(name="psum", bufs=1, space="PSUM") as psum,
        ):
            pk = const.tile([K, NF_LOC + NA], f32r)
            ones = const.tile([128, 1], bf16)
            warm = const.tile([128, 1], f32)
            bvec = const.tile([128, 1], f32)
            zeros = const.tile([128, 128 + cfg["pe_warm"][1]], bf16)
            s = const.tile([128, STREAM], bf16)
            c = const.tile([128, STREAM], bf16)
            # padded to 128 cols: a 512B/partition descriptor dodges the
            # <512B 2x DMA latency penalty; cols NJ..127 stay zero
            accs = const.tile([128, 128], f32)
            acc = psum.tile([128, NJ], f32, tag="acc")

            # zeros first: the PE warm-up below is gated on it
            nc.vector.memset(zeros[:], 0.0)
            nc.vector.memset(accs[:], 0.0)
            # sqrt-table preload during the input-DMA window
            nc.vector.memset(warm[:], 1.0)
            nc.scalar.sqrt(warm[:], warm[:])
            nc.vector.memset(ones[:], 1.0)
            nc.vector.memset(bvec[:], SQRT_BIAS)

            # staged input DMA; stage 0 carries the B cols + first atoms
            a0 = 0
            for i, aw in enumerate(cfg["dma_stages"]):
                c0 = 0 if i == 0 else NF_LOC + a0
                cw = NF_LOC + aw if i == 0 else aw
                nc.sync.dma_start(out=pk[:, c0:c0 + cw], in_=pk_d[:, c0:c0 + cw])
                a0 += aw
            assert a0 == NA

            # PE p-state warm-up during the DMA window: zero matmuls into the
            # acc bank (every column is overwritten by the ones-reduce later).
            # ~3us of continuous PE busy reaches the 2.4 GHz p-state before
            # the first real matmul issues.
            n_warm, w_warm = cfg["pe_warm"]
            for _ in range(n_warm):
                nc.tensor.matmul(
                    acc[:, 0:w_warm],
                    zeros[0:K, 0:128],
                    zeros[0:K, 128:128 + w_warm],
                    start=True, stop=True,
                )

            # grid: PE matmuls + ACT sqrt spans, round-robin over a/b tiles
            for ri, (tag, start, w) in enumerate(rounds):
                size = TA if tag == "a" else TB
                d2 = psum.tile([128, size], f32, tag=tag)
                for (pos, cw) in _chunks(start, w):
                    ft = pos // NA
                    ac_ = pos % NA
                    nc.tensor.matmul(
                        d2[:, pos - start:pos - start + cw],
                        pk[:, ft * 128:(ft + 1) * 128],
                        pk[:, NF_LOC + ac_:NF_LOC + ac_ + cw],
                        start=True, stop=True,
                    )
                nc.scalar.activation(
                    s[:, start:start + w], d2[:, 0:w],
                    mybir.ActivationFunctionType.Sqrt,
                    bias=bvec[:, 0:1],
                )

            # DVE clamp: min(s,10) then max(.,0) at 4x (bf16 SBUF->SBUF)
            bounds = cfg["dve_bounds"]
            for r0, r1 in zip(bounds[:-1], bounds[1:]):
                nc.vector.tensor_scalar(
                    c[:, r0:r1], s[:, r0:r1],
                    D_CLAMP, 0.0,
                    op0=mybir.AluOpType.min, op1=mybir.AluOpType.max,
                )

            # column sums via 1-cycle ones-matmuls. The scheduler fence keeps
            # them AFTER every grid matmul in the in-order PE queue —
            # otherwise Tile hoists them next to their DVE deps and the
            # blocked ones-matmuls stall the rest of the grid.
            tc.no_sync_barrier()
            for j in range(NJ):
                nc.tensor.matmul(
                    acc[:, j:j + 1],
                    c[:, j * 128:(j + 1) * 128],
                    ones[:, 0:1],
                    start=True, stop=True,
                )

            # Output path 1 (SP queue): the raw clamped tail slice, gated only
            # on the last DVE clamp chunk.
            nc.sync.dma_start(out=outc_d[:, :], in_=c[:, ship_from:STREAM])
            # Output path 2 (ACT queue, idle by now): PSUM has no DMA port, so
            # hop the reduced columns through SBUF, then DMA. The copy splits
            # so the bulk is gated on an early clamp chunk; only the last 16
            # columns wait for the final ones-matmuls.
            nc.scalar.copy(accs[:, 0:NJ], acc[:, 0:NJ])
            nc.scalar.dma_start(out=out_d[:, :], in_=accs[:, :])

    nc.compile()
    nc.finalize()
    return nc


def _features(predicted_frames_R, predicted_frames_t, predicted_atom_positions,
              atom_mask, true_frames_R, true_frames_t, true_atom_positions,
              seq_mask):
    """Host-side O(N+F) feature build. Returns A (NA,K), B (NF,K), counts."""
    f32 = np.float32
    Rp = np.asarray(predicted_frames_R, f32).reshape(NF, 3, 3)
    tp = np.asarray(predicted_frames_t, f32).reshape(NF, 3)
    Rt = np.asarray(true_frames_R, f32).reshape(NF, 3, 3)
    tt = np.asarray(true_frames_t, f32).reshape(NF, 3)
    p = np.asarray(predicted_atom_positions, f32).reshape(NA, 3)
    q = np.asarray(true_atom_positions, f32).reshape(NA, 3)
    m = (np.asarray(atom_mask, f32) * np.asarray(seq_mask, f32)[:, :, None]).reshape(NA)
    fm = np.broadcast_to(
        np.asarray(seq_mask, f32)[:, :, None], (B_, N_, F_)).reshape(NF)

    pp = np.einsum('aj,ak->ajk', p, p).reshape(NA, 9)
    qq = np.einsum('aj,ak->ajk', q, q).reshape(NA, 9)
    pq = np.einsum('aj,ak->ajk', p, q).reshape(NA, 9)
    Afeat = np.concatenate(
        [pp, qq, pq, p, q, np.ones((NA, 1), f32)], axis=1) * (m ** 2)[:, None]

    Gp = np.einsum('fij,fkj->fik', Rp, Rp)
    Gt = np.einsum('fij,fkj->fik', Rt, Rt)
    M = np.einsum('fij,fkj->fik', Rp, Rt)
    vec_p = -2 * np.einsum('fjk,fk->fj', Gp, tp) + 2 * np.einsum('fjk,fk->fj', M, tt)
    vec_q = -2 * np.einsum('fjk,fk->fj', Gt, tt) + 2 * np.einsum('fkj,fk->fj', M, tp)
    const = (np.einsum('fj,fjk,fk->f', tp, Gp, tp)
             + np.einsum('fj,fjk,fk->f', tt, Gt, tt)
             - 2 * np.einsum('fj,fjk,fk->f', tp, M, tt) + EPS)
    Bfeat = np.concatenate(
        [Gp.reshape(NF, 9), Gt.reshape(NF, 9), -2 * M.reshape(NF, 9),
         vec_p, vec_q, const[:, None]], axis=1)
    # fold the 0/1 frame mask into B: masked frames -> d2 == 0 -> dist 0
    Bfeat = Bfeat * fm[:, None]

    ac = max(float(m.sum()), 1.0)
    fc = max(float(fm.sum()), 1.0)
    return Afeat, Bfeat, ac, fc


def make_in_maps(inputs):
    Afeat, Bfeat, ac, fc = _features(**inputs)
    f32 = np.float32
    aT = np.ascontiguousarray(Afeat.T.astype(f32))            # (K, NA)
    in_maps = []
    for cix in range(NCORES):
        Bc = Bfeat[cix * NF_LOC:(cix + 1) * NF_LOC]           # (NF_LOC, K)
        pk = np.concatenate([Bc.T.astype(f32), aT], axis=1)   # (K, NF_LOC+NA)
        in_maps.append({"pk": np.ascontiguousarray(pk)})
    return in_maps, ac, fc


def _build_fast_exec(nc):
    """Cache the jitted 8-core executable so repeat kernel() calls skip jax
    re-tracing. Mirrors bass2jax.run_bass_via_pjrt's multi-core path."""
    import jax
    from concourse import bass2jax
    from jax.experimental.shard_map import shard_map
    from jax.sharding import Mesh, PartitionSpec

    bass2jax.install_neuronx_cc_hook()
    partition_name = nc.partition_id_tensor.name if nc.partition_id_tensor else None

    in_names, out_names, out_avals, zero_shapes = [], [], [], []
    for alloc in nc.m.functions[0].allocations:
        if not isinstance(alloc, mybir.MemoryLocationSet):
            continue
        name = alloc.memorylocations[0].name
        if alloc.kind == "ExternalInput":
            if name != partition_name:
                in_names.append(name)
        elif alloc.kind == "ExternalOutput":
            shape = tuple(alloc.tensor_shape)
            dtype = mybir.dt.np(alloc.dtype)
            out_names.append(name)
            out_avals.append(jax.core.ShapedArray(shape, dtype))
            zero_shapes.append((shape, dtype))
    n_params = len(in_names)
    all_names = in_names + out_names + ([partition_name] if partition_name else [])
    donate = tuple(range(n_params, n_params + len(out_names)))

    def _body(*args):
        operands = list(args)
        if partition_name is not None:
            operands.append(bass2jax.partition_id_tensor())
        return tuple(bass2jax._bass_exec_p.bind(
            *operands,
            out_avals=tuple(out_avals),
            in_names=tuple(all_names),
            out_names=tuple(out_names),
            lowering_input_output_aliases=(),
            sim_require_finite=True,
            sim_require_nnan=True,
            nc=nc,
        ))

    devices = jax.devices()[:NCORES]
    mesh = Mesh(np.asarray(devices), ("core",))
    specs = (PartitionSpec("core"),) * (n_params + len(out_names))
    sharded = jax.jit(
        shard_map(_body, mesh=mesh, in_specs=specs,
                  out_specs=(PartitionSpec("core"),) * len(out_names),
                  check_rep=False),
        donate_argnums=donate, keep_unused=True,
    )

    def run(in_maps):
        concat_in = [
            np.concatenate([np.asarray(m[k]) for m in in_maps], axis=0)
            for k in in_names
        ]
        concat_zeros = [
            np.zeros((NCORES * s[0], *s[1:]), dt) for (s, dt) in zero_shapes
        ]
        outs = sharded(*concat_in, *concat_zeros)
        return [
            {name: np.asarray(outs[i]).reshape(NCORES, *zero_shapes[i][0])[c]
             for i, name in enumerate(out_names)}
            for c in range(NCORES)
        ]

    return run


def kernel(**inputs) -> np.ndarray:
    in_maps, ac, fc = make_in_maps(inputs)

    if "nc" not in _cache:
        _cache["nc"] = _build_graph()
    nc = _cache["nc"]

    results = None
    try:
        if "fast" not in _cache:
            _cache["fast"] = _build_fast_exec(nc)
        results = _cache["fast"](in_maps)
    except Exception:
        _cache.pop("fast", None)
        results = run_bass_kernel_spmd(
            nc, in_maps, core_ids=list(range(NCORES))).results

    total = reduce_outputs(results)
    loss = total / (ac * fc * Z)
    return np.array([loss], np.float32)


def reduce_outputs(results) -> float:
    """Sum the per-core outputs: reduced columns + raw clamped tail slice."""
    total = 0.0
    for r in results:
        total += float(r["out"].sum(dtype=np.float64))
        total += float(np.asarray(r["outc"], np.float64).sum())
    return total
